# revision 33
# baseline (speedup 1.0000x reference)
"""Trainium2 Bass kernel for nn_CellPropagation (gnn_message_passing).

Data-parallel over 8 NeuronCores: each core handles B/8 = 128 graphs
(12800 nodes, 25600 edges).

Per-core program (all big matmuls: fp16 stationary weights x fp32r moving
operand, 1 PE cycle/row, fast weight load):
  * edge MLP 128->512->256->128 in a feature-on-partition layout;
    bias+ReLU fused into the PSUM->SBUF evacuations (split ScalarE/VectorE);
    per-graph (cell[g]+b3) accumulated into PSUM via K=1 fp16 ones-matmuls.
  * attention: q never materialized (attn = molT . (Wq@k)); full-M score
    matmuls land partition=graph; additive -1e30 mask selector; two big
    ScalarE exp ops with fused accum_out row-sums; softmax normalization
    folded into v; K=32 combine matmuls against sparse exp/param-selector
    matrices; +mol residual fused with the PSUM evacuation on VectorE.
  * attention chunks are interleaved with MLP tiles so the PE stream is
    dense from t=0 (HAM stays warm) and no phase barrier exists.

Host side (inside kernel()): shard + transpose inputs, build the selector
matrices, run SPMD via run_bass_kernel_spmd, transpose outputs back.
"""

import numpy as np

import concourse.bass as bass
import concourse.bacc as bacc
from concourse import mybir
from concourse.tile import TileContext
from concourse.bass_utils import run_bass_kernel_spmd

# ---------------- problem constants ----------------
B, N, D = 1024, 100, 128
E_PER = 200
H1, H2 = 512, 256
CORES = 8
B_LOC = B // CORES            # 128 graphs / core
SQRT_D = float(np.sqrt(D))

F32 = mybir.dt.float32
F32R = mybir.dt.float32r
F16 = mybir.dt.float16
AF = mybir.ActivationFunctionType
ALU = mybir.AluOpType
AX = mybir.AxisListType

ET = 512                      # edges per MLP tile
EDMA = 2                      # MLP tiles per edge-input DMA
ACH = 400                     # attention chunk columns (= 4 graphs)
NEG = np.float32(-1e30)


def build_nc(b_loc=B_LOC):
    """Build the per-core bass program (same program on every core)."""
    n_nodes = b_loc * N
    n_edges = b_loc * E_PER
    gblk = min(32, b_loc)             # graphs per attention block
    assert b_loc % gblk == 0 and (gblk * N) % ACH == 0
    nblk = b_loc // gblk
    cblk = gblk * N                   # cols per attention block (3200)
    nch = cblk // ACH                 # chunks per block (8)
    nchunks = nblk * nch              # total attention chunks (32)
    n_et = (n_edges + ET - 1) // ET   # MLP tiles (50)
    assert n_edges % ET == 0

    nc = bacc.Bacc("TRN2", target_bir_lowering=False, debug=False)

    def din(name, shape, dt=F32):
        return nc.dram_tensor(name, list(shape), dt, kind="ExternalInput").ap()

    CP32 = H1 + 5 * D + b_loc + (H1 // D) + (H2 // D) + 1
    CP16 = (H1 // D) * H2 + (H2 // D) * D
    edgeT = din("edgeT", [D, n_edges], F32R)
    xT_d = din("xT", [D, n_nodes], F32R)
    cp32_d = din("cp32", [D, CP32], F32R)
    cp16_d = din("cp16", [D, CP16], F16)
    cp1_d = din("cp1", [1, D], F16)
    cellsegT = din("cellsegT", [D, n_edges])
    Msel_d = din("Msel", [b_loc, cblk])
    PMsel_d = din("PMsel", [b_loc, cblk], F32R)

    newxT = nc.dram_tensor("newxT", [D, n_nodes], F32, kind="ExternalOutput").ap()
    neweT = nc.dram_tensor("neweT", [D, n_edges], F32, kind="ExternalOutput").ap()

    with TileContext(nc) as tc:
        with (
            tc.tile_pool(name="persist", bufs=1) as cp,
            tc.tile_pool(name="mlp_in", bufs=4) as ep,
            tc.tile_pool(name="mlp_h1", bufs=3) as h1p,
            tc.tile_pool(name="mlp_h2", bufs=3) as h2p,
            tc.tile_pool(name="mlp_out", bufs=3) as nep,
            tc.tile_pool(name="nx_out", bufs=2) as nxp,
            tc.tile_pool(name="p1", bufs=1, space="PSUM") as p1p,
            tc.tile_pool(name="p2", bufs=1, space="PSUM") as p2p,
            tc.tile_pool(name="pout", bufs=1, space="PSUM") as pop,
        ):
            def cload(dap, shape, dt=None):
                dt = dt or dap.dtype
                t = cp.tile(list(shape), dt, tag=dap.tensor.name,
                            name=f"c_{dap.tensor.name}")
                nc.sync.dma_start(out=t[:, :], in_=dap)
                return t

            cp32 = cp.tile([D, CP32], F32R, tag="cp32", name="c_cp32")
            SPLIT = 5 * D + b_loc + (H1 // D) + (H2 // D) + 1
            nc.sync.dma_start(out=cp32[:, :SPLIT], in_=cp32_d[:, :SPLIT])
            nc.sync.dma_start(out=cp32[:, SPLIT:], in_=cp32_d[:, SPLIT:])
            cp16 = cload(cp16_d, [D, CP16])
            cp1 = cload(cp1_d, [1, D], F16)
            o = [0]

            def csl(src, n):
                v = src[:, o[0]:o[0] + n]
                o[0] += n
                return v

            Wk = csl(cp32, D)
            WqT = csl(cp32, D)
            Wvs = csl(cp32, D)
            cellT = csl(cp32, b_loc)
            cellR = csl(cp32, D)
            b1c = csl(cp32, H1 // D).bitcast(F32)
            b2c = csl(cp32, H2 // D).bitcast(F32)
            bkc = csl(cp32, 1).bitcast(F32)
            W1 = csl(cp32, H1)
            o = [0]
            W2r = csl(cp16, (H1 // D) * H2)
            W3r = csl(cp16, (H2 // D) * D)
            bvs = cp1[:, 0:D]

            ones1 = cp.tile([1, b_loc], F16, tag="ones1")
            nc.vector.memset(ones1[:, :], 1.0)

            molT = cp.tile([D, n_nodes], F32R, tag="molT")
            Xm = cp.tile([b_loc, cblk], F32, tag="Xm")
            E = cp.tile([b_loc, cblk], F32R, tag="E")
            rs2 = cp.tile([b_loc, 2], F32, tag="rs2")
            rs = cp.tile([b_loc, 1], F32, tag="rs")
            rcp = cp.tile([b_loc, 1], F32, tag="rcp")
            kT = cp.tile([D, b_loc], F32R, tag="kT")
            KQ = cp.tile([D, b_loc], F32R, tag="KQ")
            vraw = cp.tile([b_loc, D], F32, tag="vraw")
            vs = cp.tile([b_loc, D], F32R, tag="vs")

            # ---------- attention pre: k, KQ = Wq@k, v (tiny fp32r MMs) --
            pk = p2p.tile([D, b_loc], F32, tag="p2_0", name="pk")
            nc.tensor.matmul(pk[:, :], Wk[:, :], cellT[:, :])
            nc.vector.tensor_scalar_add(kT[:, :], pk[:, :], bkc[:, 0:1])

            pq = p2p.tile([D, b_loc], F32, tag="p2_0", name="pq")
            nc.tensor.matmul(pq[:, :], WqT[:, :], kT[:, :])
            nc.vector.tensor_copy(KQ[:, :], pq[:, :])

            pv = p2p.tile([b_loc, D], F32, tag="p2_1", name="pv")
            nc.tensor.matmul(pv[:, :], cellT[:, :], Wvs[:, :],
                             start=True, stop=False)
            nc.tensor.matmul(pv[:, :], ones1[:, :], bvs[:, :],
                             start=False, stop=True)
            nc.vector.tensor_copy(vraw[:, :], pv[:, :])

            # ---------- per-chunk attention scores (interleaved) ---------
            def attn_chunk(i):
                b, ch = divmod(i, nch)
                lo = b * cblk + ch * ACH
                nc.sync.dma_start(out=molT[:, lo:lo + ACH],
                                  in_=xT_d[:, lo:lo + ACH])
                pa = pop.tile([b_loc, ACH], F32, tag="pa", name="pa", bufs=1)
                nc.tensor.matmul(pa[:, :], KQ[:, :], molT[:, lo:lo + ACH])
                sl = slice(b * gblk, (b + 1) * gblk)
                cs = slice(ch * ACH, (ch + 1) * ACH)
                nc.vector.tensor_tensor(Xm[sl, cs], pa[sl, :], Msel[sl, cs],
                                        ALU.add)

            # ---------- MLP tile (two pipeline stages) -------------------
            eg = [None]
            csd = {}
            h1s = {}

            def mlp_l1(t):
                gi = t % EDMA
                if gi == 0:
                    w = min(EDMA, n_et - t) * ET
                    eg[0] = ep.tile([D, EDMA * ET], F32R, tag="eg", name="eg")
                    nc.sync.dma_start(out=eg[0][:, :w],
                                      in_=edgeT[:, t * ET:t * ET + w])
                    csd[t // EDMA] = ep.tile([D, EDMA * ET], F32,
                                             tag="cs", name="cs")
                    nc.sync.dma_start(out=csd[t // EDMA][:, :w],
                                      in_=cellsegT[:, t * ET:t * ET + w])
                eT = eg[0][:, gi * ET:(gi + 1) * ET]

                # L1: 128 -> 512 (+b1, relu)
                p1 = [p1p.tile([D, ET], F32, tag=f"p1_{m}", name=f"p1_{m}")
                      for m in range(4)]
                for m in range(4):
                    nc.tensor.matmul(p1[m][:, :],
                                     W1[:, m * D:(m + 1) * D], eT)
                h1 = h1p.tile([D, 4 * ET], F16, tag="h1", name="h1")
                h1s[t] = h1
                for m in range(4):
                    dst = h1[:, m * ET:(m + 1) * ET]
                    src = p1[m][:, :]
                    if m < 3:
                        nc.scalar.activation(dst, src, AF.Relu,
                                             bias=b1c[:, m:m + 1])
                    else:
                        nc.vector.tensor_scalar(dst, src, b1c[:, m:m + 1],
                                                0.0, ALU.add, ALU.max)

            def mlp_rest(t):
                h1 = h1s.pop(t)
                # L2: 512 -> 256 (+b2, relu)
                p2 = [p2p.tile([D, ET], F32, tag=f"p2_{m}", name=f"p2_{m}")
                      for m in range(2)]
                for m in range(2):
                    for k in range(4):
                        nc.tensor.matmul(
                            p2[m][:, :],
                            W2r[:, k * H2 + m * D:k * H2 + (m + 1) * D],
                            h1[:, k * ET:(k + 1) * ET],
                            start=(k == 0), stop=(k == 3))
                h2 = h2p.tile([D, 2 * ET], F16, tag="h2", name="h2")
                for m in range(2):
                    dst = h2[:, m * ET:(m + 1) * ET]
                    src = p2[m][:, :]
                    if m == 0 and t % 3 != 0:
                        nc.scalar.activation(dst, src, AF.Relu,
                                             bias=b2c[:, m:m + 1])
                    else:
                        nc.vector.tensor_scalar(dst, src, b2c[:, m:m + 1],
                                                0.0, ALU.add, ALU.max)

                # L3: 256 -> 128; (cell[g] + b3) added during the evacuation
                p3 = pop.tile([D, ET], F32, tag="p3", name="p3")
                nc.tensor.matmul(p3[:, :], W3r[:, 0:D], h2[:, 0:ET],
                                 start=True, stop=False)
                nc.tensor.matmul(p3[:, :], W3r[:, D:2 * D], h2[:, ET:2 * ET],
                                 start=False, stop=True)

                oi = t % 2
                if oi == 0:
                    mlp_rest.ne = nep.tile([D, 2 * ET], F32, tag="ne",
                                           name="ne")
                ne = mlp_rest.ne
                dst = ne[:, oi * ET:(oi + 1) * ET]
                cs = csd[t // EDMA]
                if t % EDMA == EDMA - 1 or t == n_et - 1:
                    del csd[t // EDMA]
                nc.vector.tensor_tensor(
                    dst, p3[:, :], cs[:, (t % EDMA) * ET:(t % EDMA + 1) * ET],
                    ALU.add)
                if oi == 1 or t == n_et - 1:
                    w = (oi + 1) * ET
                    base = (t - oi) * ET
                    nc.gpsimd.dma_start(out=neweT[:, base:base + w],
                                        in_=ne[:, :w])

            # ---------- softmax tail + combine helpers -------------------
            def softmax_tail():
                half = cblk // 2
                for j in range(2):
                    nc.scalar.activation(E[:, j * half:(j + 1) * half],
                                         Xm[:, j * half:(j + 1) * half],
                                         AF.Exp, accum_out=rs2[:, j:j + 1])
                nc.vector.reduce_sum(rs[:, :], rs2[:, :], axis=AX.X)
                nc.vector.reciprocal(rcp[:, :], rs[:, :])
                nc.vector.tensor_scalar_mul(vs[:, :], vraw[:, :], rcp[:, 0:1])

            nxs = [None]

            def combine_chunk(i):
                b, ch = divmod(i, nch)
                sl = slice(b * gblk, (b + 1) * gblk)
                tp = (b * gblk, 0)
                cs = slice(ch * ACH, (ch + 1) * ACH)
                pc = pop.tile([D, ACH], F32,
                              tag=("pa" if i % 2 else "p3"), name="pc")
                nc.tensor.matmul(pc[:, :], vs[sl, :], E[sl, cs],
                                 start=True, stop=False, tile_position=tp)
                nc.tensor.matmul(pc[:, :], cellR[sl, :], PMsel[sl, cs],
                                 start=False, stop=True, tile_position=tp)
                oc = i % 2
                if oc == 0:
                    nxs[0] = nxp.tile([D, 2 * ACH], F32, tag="nx", name="nx")
                off = b * cblk + ch * ACH
                nc.vector.tensor_tensor(
                    nxs[0][:, oc * ACH:(oc + 1) * ACH], pc[:, :],
                    molT[:, off:off + ACH].bitcast(F32), ALU.add)
                if oc == 1 or i == nchunks - 1:
                    w = (oc + 1) * ACH
                    base = (i - oc) * ACH
                    nc.gpsimd.dma_start(out=newxT[:, base:base + w],
                                        in_=nxs[0][:, :w])

            # ---------- main interleaved loop (1-deep SW pipeline) -------
            cstart = nchunks + 3            # first tile carrying combines
            ncomb = max(1, n_et - cstart)
            mlp_l1(0)
            Msel = cload(Msel_d, [b_loc, cblk])
            PMsel = cload(PMsel_d, [b_loc, cblk])
            for t in range(n_et):
                if t + 1 < n_et:
                    mlp_l1(t + 1)
                if t < nchunks:
                    attn_chunk(t)
                mlp_rest(t)
                if t == nchunks:
                    softmax_tail()
                if t >= cstart:
                    lo = (t - cstart) * nchunks // ncomb
                    hi = (t - cstart + 1) * nchunks // ncomb
                    for i in range(lo, hi):
                        combine_chunk(i)
    nc.compile()
    return nc


# ---------------- host-side sharding / prep ----------------

def make_in_map(x, cell, mask, edge_attr,
                Wq, bq, Wk, bk, Wv, bv,
                W1, b1, W2, b2, W3, b3, param, b_loc):
    """Build the per-core input map from this core's shard (numpy, fp32)."""
    gblk = min(32, b_loc)
    cblk = gblk * N
    f = np.float32
    h = np.float16
    notm = (~mask).astype(f)                       # [b_loc, N]
    msel = np.full((b_loc, cblk), NEG, f)
    pmsel = np.zeros((b_loc, cblk), f)
    for g in range(b_loc):
        base = (g % gblk) * N
        msel[g, base:base + N] = np.where(mask[g], NEG, f(0.0))
        pmsel[g, base:base + N] = param.astype(f) * notm[g]

    W2r = np.concatenate([W2[k * D:(k + 1) * D, :] for k in range(H1 // D)],
                         axis=1)
    W3r = np.concatenate([W3[k * D:(k + 1) * D, :] for k in range(H2 // D)],
                         axis=1)
    cellR = np.zeros((D, D), f)
    cellR[:b_loc, :] = cell
    CP32 = H1 + 5 * D + b_loc + (H1 // D) + (H2 // D) + 1
    cp32 = np.zeros((D, CP32), f)
    off = 0
    for blk in (Wk, Wq.T, Wv / SQRT_D, cell.T, cellR,
                b1.reshape(H1 // D, D).T, b2.reshape(H2 // D, D).T,
                bk.reshape(D, 1), W1):
        blk = np.asarray(blk, f)
        cp32[:blk.shape[0], off:off + blk.shape[1]] = blk
        off += blk.shape[1]
    cp16 = np.concatenate([W2r, W3r], axis=1).astype(h)
    cp1 = (bv / SQRT_D).astype(h).reshape(1, D)
    seg = np.repeat(np.arange(b_loc), E_PER)
    cellseg = np.ascontiguousarray((cell[seg] + b3[None, :]).T.astype(f))
    return {
        "edgeT": np.ascontiguousarray(edge_attr.T),
        "xT": np.ascontiguousarray(x.T),
        "cp32": cp32,
        "cp16": np.ascontiguousarray(cp16),
        "cp1": np.ascontiguousarray(cp1),
        "cellsegT": cellseg,
        "Msel": msel,
        "PMsel": pmsel,
    }


def make_all_in_maps(inputs, b_loc=B_LOC, n_cores=CORES):
    inp = {k: np.asarray(v) for k, v in inputs.items()}
    num_edge = inp["num_edge"]
    assert int(num_edge.min()) == E_PER and int(num_edge.max()) == E_PER, \
        "kernel assumes a constant 200 edges per graph"
    f = np.float32
    maps = []
    for c in range(n_cores):
        gs = slice(c * b_loc, (c + 1) * b_loc)
        ns = slice(c * b_loc * N, (c + 1) * b_loc * N)
        es = slice(c * b_loc * E_PER, (c + 1) * b_loc * E_PER)
        maps.append(make_in_map(
            inp["x"][ns].astype(f), inp["cell"][gs].astype(f),
            np.asarray(inp["mask"][gs]).astype(bool),
            inp["edge_attr"][es].astype(f),
            inp["Wq"].astype(f), inp["bq"].astype(f),
            inp["Wk"].astype(f), inp["bk"].astype(f),
            inp["Wv"].astype(f), inp["bv"].astype(f),
            inp["W1"].astype(f), inp["b1"].astype(f),
            inp["W2"].astype(f), inp["b2"].astype(f),
            inp["W3"].astype(f), inp["b3"].astype(f),
            inp["param"].astype(f), b_loc))
    return maps


_NC_CACHE = {}


def _get_nc(b_loc=B_LOC):
    if b_loc not in _NC_CACHE:
        _NC_CACHE[b_loc] = build_nc(b_loc)
    return _NC_CACHE[b_loc]


def run(inputs, trace=False, **kw):
    """Run on 8 NeuronCores; returns ((new_x, new_edge_attr), results)."""
    nc = _get_nc()
    in_maps = make_all_in_maps(inputs)
    res = run_bass_kernel_spmd(nc, in_maps, list(range(CORES)),
                               trace=trace, **kw)
    new_x = np.concatenate(
        [np.ascontiguousarray(res.results[c]["newxT"].T)
         for c in range(CORES)], axis=0)
    new_e = np.concatenate(
        [np.ascontiguousarray(res.results[c]["neweT"].T)
         for c in range(CORES)], axis=0)
    return (new_x, new_e), res


def kernel(**inputs):
    (new_x, new_e), _ = run(inputs, trace=False)
    return new_x, new_e


# revision 35
# speedup vs baseline: 1.0232x; 1.0232x over previous
"""Trainium2 Bass kernel for nn_CellPropagation (gnn_message_passing).

Data-parallel over 8 NeuronCores: each core handles B/8 = 128 graphs
(12800 nodes, 25600 edges).

Per-core program (all big matmuls: fp16 stationary weights x fp32r moving
operand, 1 PE cycle/row, fast weight load):
  * edge MLP 128->512->256->128 in a feature-on-partition layout;
    bias+ReLU fused into the PSUM->SBUF evacuations (split ScalarE/VectorE);
    per-graph (cell[g]+b3) accumulated into PSUM via K=1 fp16 ones-matmuls.
  * attention: q never materialized (attn = molT . (Wq@k)); full-M score
    matmuls land partition=graph; additive -1e30 mask selector; two big
    ScalarE exp ops with fused accum_out row-sums; softmax normalization
    folded into v; K=32 combine matmuls against sparse exp/param-selector
    matrices; +mol residual fused with the PSUM evacuation on VectorE.
  * attention chunks are interleaved with MLP tiles so the PE stream is
    dense from t=0 (HAM stays warm) and no phase barrier exists.

Host side (inside kernel()): shard + transpose inputs, build the selector
matrices, run SPMD via run_bass_kernel_spmd, transpose outputs back.
"""

import numpy as np

import concourse.bass as bass
import concourse.bacc as bacc
from concourse import mybir
from concourse.tile import TileContext
from concourse.bass_utils import run_bass_kernel_spmd

# ---------------- problem constants ----------------
B, N, D = 1024, 100, 128
E_PER = 200
H1, H2 = 512, 256
CORES = 8
B_LOC = B // CORES            # 128 graphs / core
SQRT_D = float(np.sqrt(D))

F32 = mybir.dt.float32
F32R = mybir.dt.float32r
F16 = mybir.dt.float16
AF = mybir.ActivationFunctionType
ALU = mybir.AluOpType
AX = mybir.AxisListType

ET = 512                      # edges per MLP tile
EDMA = 2                      # MLP tiles per edge-input DMA
ACH = 400                     # attention chunk columns (= 4 graphs)
NEG = np.float32(-1e30)


def build_nc(b_loc=B_LOC):
    """Build the per-core bass program (same program on every core)."""
    n_nodes = b_loc * N
    n_edges = b_loc * E_PER
    gblk = min(32, b_loc)             # graphs per attention block
    assert b_loc % gblk == 0 and (gblk * N) % ACH == 0
    nblk = b_loc // gblk
    cblk = gblk * N                   # cols per attention block (3200)
    nch = cblk // ACH                 # chunks per block (8)
    nchunks = nblk * nch              # total attention chunks (32)
    n_et = (n_edges + ET - 1) // ET   # MLP tiles (50)
    assert n_edges % ET == 0

    nc = bacc.Bacc("TRN2", target_bir_lowering=False, debug=False)

    def din(name, shape, dt=F32):
        return nc.dram_tensor(name, list(shape), dt, kind="ExternalInput").ap()

    CP32 = H1 + 5 * D + b_loc + (H1 // D) + (H2 // D) + 1
    CP16 = (H1 // D) * H2 + (H2 // D) * D
    edgeT = din("edgeT", [D, n_edges], F32R)
    xT_d = din("xT", [D, n_nodes], F32R)
    cp32_d = din("cp32", [D, CP32], F32R)
    cp16_d = din("cp16", [D, CP16], F16)
    cp1_d = din("cp1", [1, D], F16)
    cellsegT = din("cellsegT", [D, n_edges])
    Msel_d = din("Msel", [b_loc, cblk])
    PMsel_d = din("PMsel", [b_loc, cblk], F32R)

    newxT = nc.dram_tensor("newxT", [D, n_nodes], F32, kind="ExternalOutput").ap()
    neweT = nc.dram_tensor("neweT", [D, n_edges], F32, kind="ExternalOutput").ap()

    with TileContext(nc) as tc:
        with (
            tc.tile_pool(name="persist", bufs=1) as cp,
            tc.tile_pool(name="mlp_in", bufs=4) as ep,
            tc.tile_pool(name="mlp_h1", bufs=3) as h1p,
            tc.tile_pool(name="mlp_h2", bufs=4) as h2p,
            tc.tile_pool(name="mlp_out", bufs=3) as nep,
            tc.tile_pool(name="nx_out", bufs=3) as nxp,
            tc.tile_pool(name="p1", bufs=1, space="PSUM") as p1p,
            tc.tile_pool(name="p2", bufs=1, space="PSUM") as p2p,
            tc.tile_pool(name="pout", bufs=1, space="PSUM") as pop,
        ):
            def cload(dap, shape, dt=None):
                dt = dt or dap.dtype
                t = cp.tile(list(shape), dt, tag=dap.tensor.name,
                            name=f"c_{dap.tensor.name}")
                nc.sync.dma_start(out=t[:, :], in_=dap)
                return t

            cp32 = cp.tile([D, CP32], F32R, tag="cp32", name="c_cp32")
            SPLIT = 5 * D + b_loc + (H1 // D) + (H2 // D) + 1
            nc.sync.dma_start(out=cp32[:, :SPLIT], in_=cp32_d[:, :SPLIT])
            nc.sync.dma_start(out=cp32[:, SPLIT:], in_=cp32_d[:, SPLIT:])
            cp16 = cload(cp16_d, [D, CP16])
            cp1 = cload(cp1_d, [1, D], F16)
            o = [0]

            def csl(src, n):
                v = src[:, o[0]:o[0] + n]
                o[0] += n
                return v

            Wk = csl(cp32, D)
            WqT = csl(cp32, D)
            Wvs = csl(cp32, D)
            cellT = csl(cp32, b_loc)
            cellR = csl(cp32, D)
            b1c = csl(cp32, H1 // D).bitcast(F32)
            b2c = csl(cp32, H2 // D).bitcast(F32)
            bkc = csl(cp32, 1).bitcast(F32)
            W1 = csl(cp32, H1)
            o = [0]
            W2r = csl(cp16, (H1 // D) * H2)
            W3r = csl(cp16, (H2 // D) * D)
            bvs = cp1[:, 0:D]

            ones1 = cp.tile([1, b_loc], F16, tag="ones1")
            nc.vector.memset(ones1[:, :], 1.0)

            molT = cp.tile([D, n_nodes], F32R, tag="molT")
            Xm = cp.tile([b_loc, cblk], F32, tag="Xm")
            E = cp.tile([b_loc, cblk], F32R, tag="E")
            rs2 = cp.tile([b_loc, 2], F32, tag="rs2")
            rs = cp.tile([b_loc, 1], F32, tag="rs")
            rcp = cp.tile([b_loc, 1], F32, tag="rcp")
            kT = cp.tile([D, b_loc], F32R, tag="kT")
            KQ = cp.tile([D, b_loc], F32R, tag="KQ")
            vraw = cp.tile([b_loc, D], F32, tag="vraw")
            vs = cp.tile([b_loc, D], F32R, tag="vs")

            # ---------- attention pre: k, KQ = Wq@k, v (tiny fp32r MMs) --
            pk = p2p.tile([D, b_loc], F32, tag="p2_0", name="pk")
            nc.tensor.matmul(pk[:, :], Wk[:, :], cellT[:, :])
            nc.vector.tensor_scalar_add(kT[:, :], pk[:, :], bkc[:, 0:1])

            pq = p2p.tile([D, b_loc], F32, tag="p2_0", name="pq")
            nc.tensor.matmul(pq[:, :], WqT[:, :], kT[:, :])
            nc.vector.tensor_copy(KQ[:, :], pq[:, :])

            pv = p2p.tile([b_loc, D], F32, tag="p2_1", name="pv")
            nc.tensor.matmul(pv[:, :], cellT[:, :], Wvs[:, :],
                             start=True, stop=False)
            nc.tensor.matmul(pv[:, :], ones1[:, :], bvs[:, :],
                             start=False, stop=True)
            nc.vector.tensor_copy(vraw[:, :], pv[:, :])

            # ---------- per-chunk attention scores (interleaved) ---------
            def attn_chunk(i):
                b, ch = divmod(i, nch)
                lo = b * cblk + ch * ACH
                nc.sync.dma_start(out=molT[:, lo:lo + ACH],
                                  in_=xT_d[:, lo:lo + ACH])
                pa = pop.tile([b_loc, ACH], F32, tag="pa", name="pa", bufs=1)
                nc.tensor.matmul(pa[:, :], KQ[:, :], molT[:, lo:lo + ACH])
                sl = slice(b * gblk, (b + 1) * gblk)
                cs = slice(ch * ACH, (ch + 1) * ACH)
                nc.vector.tensor_tensor(Xm[sl, cs], pa[sl, :], Msel[sl, cs],
                                        ALU.add)

            # ---------- MLP tile (two pipeline stages) -------------------
            eg = [None]
            csd = {}
            h1s = {}

            def mlp_l1(t):
                gi = t % EDMA
                if gi == 0:
                    w = min(EDMA, n_et - t) * ET
                    eg[0] = ep.tile([D, EDMA * ET], F32R, tag="eg", name="eg")
                    nc.sync.dma_start(out=eg[0][:, :w],
                                      in_=edgeT[:, t * ET:t * ET + w])
                    csd[t // EDMA] = ep.tile([D, EDMA * ET], F32,
                                             tag="cs", name="cs")
                    nc.sync.dma_start(out=csd[t // EDMA][:, :w],
                                      in_=cellsegT[:, t * ET:t * ET + w])
                eT = eg[0][:, gi * ET:(gi + 1) * ET]

                # L1: 128 -> 512 (+b1, relu)
                p1 = [p1p.tile([D, ET], F32, tag=f"p1_{m}", name=f"p1_{m}")
                      for m in range(4)]
                for m in range(4):
                    nc.tensor.matmul(p1[m][:, :],
                                     W1[:, m * D:(m + 1) * D], eT)
                h1 = h1p.tile([D, 4 * ET], F16, tag="h1", name="h1")
                h1s[t] = h1
                for m in range(4):
                    dst = h1[:, m * ET:(m + 1) * ET]
                    src = p1[m][:, :]
                    if m < 3:
                        nc.scalar.activation(dst, src, AF.Relu,
                                             bias=b1c[:, m:m + 1])
                    else:
                        nc.vector.tensor_scalar(dst, src, b1c[:, m:m + 1],
                                                0.0, ALU.add, ALU.max)

            def mlp_rest(t):
                h1 = h1s.pop(t)
                # L2: 512 -> 256 (+b2, relu)
                p2 = [p2p.tile([D, ET], F32, tag=f"p2_{m}", name=f"p2_{m}")
                      for m in range(2)]
                for m in range(2):
                    for k in range(4):
                        nc.tensor.matmul(
                            p2[m][:, :],
                            W2r[:, k * H2 + m * D:k * H2 + (m + 1) * D],
                            h1[:, k * ET:(k + 1) * ET],
                            start=(k == 0), stop=(k == 3))
                h2 = h2p.tile([D, 2 * ET], F16, tag="h2", name="h2")
                for m in range(2):
                    dst = h2[:, m * ET:(m + 1) * ET]
                    src = p2[m][:, :]
                    if m == 0 and t % 3 != 0:
                        nc.scalar.activation(dst, src, AF.Relu,
                                             bias=b2c[:, m:m + 1])
                    else:
                        nc.vector.tensor_scalar(dst, src, b2c[:, m:m + 1],
                                                0.0, ALU.add, ALU.max)

                # L3: 256 -> 128; (cell[g] + b3) added during the evacuation
                p3 = pop.tile([D, ET], F32, tag="p3", name="p3")
                nc.tensor.matmul(p3[:, :], W3r[:, 0:D], h2[:, 0:ET],
                                 start=True, stop=False)
                nc.tensor.matmul(p3[:, :], W3r[:, D:2 * D], h2[:, ET:2 * ET],
                                 start=False, stop=True)

                oi = t % 2
                if oi == 0:
                    mlp_rest.ne = nep.tile([D, 2 * ET], F32, tag="ne",
                                           name="ne")
                ne = mlp_rest.ne
                dst = ne[:, oi * ET:(oi + 1) * ET]
                cs = csd[t // EDMA]
                if t % EDMA == EDMA - 1 or t == n_et - 1:
                    del csd[t // EDMA]
                nc.vector.tensor_tensor(
                    dst, p3[:, :], cs[:, (t % EDMA) * ET:(t % EDMA + 1) * ET],
                    ALU.add)
                if oi == 1 or t == n_et - 1:
                    w = (oi + 1) * ET
                    base = (t - oi) * ET
                    nc.gpsimd.dma_start(out=neweT[:, base:base + w],
                                        in_=ne[:, :w])

            # ---------- softmax tail + combine helpers -------------------
            def softmax_tail():
                half = cblk // 2
                for j in range(2):
                    nc.scalar.activation(E[:, j * half:(j + 1) * half],
                                         Xm[:, j * half:(j + 1) * half],
                                         AF.Exp, accum_out=rs2[:, j:j + 1])
                nc.vector.reduce_sum(rs[:, :], rs2[:, :], axis=AX.X)
                nc.vector.reciprocal(rcp[:, :], rs[:, :])
                nc.vector.tensor_scalar_mul(vs[:, :], vraw[:, :], rcp[:, 0:1])

            nxs = [None]

            def combine_chunk(i):
                b, ch = divmod(i, nch)
                sl = slice(b * gblk, (b + 1) * gblk)
                tp = (b * gblk, 0)
                cs = slice(ch * ACH, (ch + 1) * ACH)
                pc = pop.tile([D, ACH], F32,
                              tag=("pa" if i % 2 else "p3"), name="pc")
                nc.tensor.matmul(pc[:, :], vs[sl, :], E[sl, cs],
                                 start=True, stop=False, tile_position=tp)
                nc.tensor.matmul(pc[:, :], cellR[sl, :], PMsel[sl, cs],
                                 start=False, stop=True, tile_position=tp)
                oc = i % 2
                if oc == 0:
                    nxs[0] = nxp.tile([D, 2 * ACH], F32, tag="nx", name="nx")
                off = b * cblk + ch * ACH
                nc.vector.tensor_tensor(
                    nxs[0][:, oc * ACH:(oc + 1) * ACH], pc[:, :],
                    molT[:, off:off + ACH].bitcast(F32), ALU.add)
                if oc == 1 or i == nchunks - 1:
                    w = (oc + 1) * ACH
                    base = (i - oc) * ACH
                    nc.gpsimd.dma_start(out=newxT[:, base:base + w],
                                        in_=nxs[0][:, :w])

            # ---------- main interleaved loop (1-deep SW pipeline) -------
            cstart = nchunks + 3            # first tile carrying combines
            ncomb = max(1, n_et - cstart)
            mlp_l1(0)
            Msel = cload(Msel_d, [b_loc, cblk])
            PMsel = cload(PMsel_d, [b_loc, cblk])
            for t in range(n_et):
                if t + 1 < n_et:
                    mlp_l1(t + 1)
                mlp_rest(t)
                if t < nchunks:
                    attn_chunk(t)
                if t == nchunks:
                    softmax_tail()
                if t >= cstart:
                    lo = (t - cstart) * nchunks // ncomb
                    hi = (t - cstart + 1) * nchunks // ncomb
                    for i in range(lo, hi):
                        combine_chunk(i)
    nc.compile()
    return nc


# ---------------- host-side sharding / prep ----------------

def make_in_map(x, cell, mask, edge_attr,
                Wq, bq, Wk, bk, Wv, bv,
                W1, b1, W2, b2, W3, b3, param, b_loc):
    """Build the per-core input map from this core's shard (numpy, fp32)."""
    gblk = min(32, b_loc)
    cblk = gblk * N
    f = np.float32
    h = np.float16
    notm = (~mask).astype(f)                       # [b_loc, N]
    msel = np.full((b_loc, cblk), NEG, f)
    pmsel = np.zeros((b_loc, cblk), f)
    for g in range(b_loc):
        base = (g % gblk) * N
        msel[g, base:base + N] = np.where(mask[g], NEG, f(0.0))
        pmsel[g, base:base + N] = param.astype(f) * notm[g]

    W2r = np.concatenate([W2[k * D:(k + 1) * D, :] for k in range(H1 // D)],
                         axis=1)
    W3r = np.concatenate([W3[k * D:(k + 1) * D, :] for k in range(H2 // D)],
                         axis=1)
    cellR = np.zeros((D, D), f)
    cellR[:b_loc, :] = cell
    CP32 = H1 + 5 * D + b_loc + (H1 // D) + (H2 // D) + 1
    cp32 = np.zeros((D, CP32), f)
    off = 0
    for blk in (Wk, Wq.T, Wv / SQRT_D, cell.T, cellR,
                b1.reshape(H1 // D, D).T, b2.reshape(H2 // D, D).T,
                bk.reshape(D, 1), W1):
        blk = np.asarray(blk, f)
        cp32[:blk.shape[0], off:off + blk.shape[1]] = blk
        off += blk.shape[1]
    cp16 = np.concatenate([W2r, W3r], axis=1).astype(h)
    cp1 = (bv / SQRT_D).astype(h).reshape(1, D)
    seg = np.repeat(np.arange(b_loc), E_PER)
    cellseg = np.ascontiguousarray((cell[seg] + b3[None, :]).T.astype(f))
    return {
        "edgeT": np.ascontiguousarray(edge_attr.T),
        "xT": np.ascontiguousarray(x.T),
        "cp32": cp32,
        "cp16": np.ascontiguousarray(cp16),
        "cp1": np.ascontiguousarray(cp1),
        "cellsegT": cellseg,
        "Msel": msel,
        "PMsel": pmsel,
    }


def make_all_in_maps(inputs, b_loc=B_LOC, n_cores=CORES):
    inp = {k: np.asarray(v) for k, v in inputs.items()}
    num_edge = inp["num_edge"]
    assert int(num_edge.min()) == E_PER and int(num_edge.max()) == E_PER, \
        "kernel assumes a constant 200 edges per graph"
    f = np.float32
    maps = []
    for c in range(n_cores):
        gs = slice(c * b_loc, (c + 1) * b_loc)
        ns = slice(c * b_loc * N, (c + 1) * b_loc * N)
        es = slice(c * b_loc * E_PER, (c + 1) * b_loc * E_PER)
        maps.append(make_in_map(
            inp["x"][ns].astype(f), inp["cell"][gs].astype(f),
            np.asarray(inp["mask"][gs]).astype(bool),
            inp["edge_attr"][es].astype(f),
            inp["Wq"].astype(f), inp["bq"].astype(f),
            inp["Wk"].astype(f), inp["bk"].astype(f),
            inp["Wv"].astype(f), inp["bv"].astype(f),
            inp["W1"].astype(f), inp["b1"].astype(f),
            inp["W2"].astype(f), inp["b2"].astype(f),
            inp["W3"].astype(f), inp["b3"].astype(f),
            inp["param"].astype(f), b_loc))
    return maps


_NC_CACHE = {}


def _get_nc(b_loc=B_LOC):
    if b_loc not in _NC_CACHE:
        _NC_CACHE[b_loc] = build_nc(b_loc)
    return _NC_CACHE[b_loc]


def run(inputs, trace=False, **kw):
    """Run on 8 NeuronCores; returns ((new_x, new_edge_attr), results)."""
    nc = _get_nc()
    in_maps = make_all_in_maps(inputs)
    res = run_bass_kernel_spmd(nc, in_maps, list(range(CORES)),
                               trace=trace, **kw)
    new_x = np.concatenate(
        [np.ascontiguousarray(res.results[c]["newxT"].T)
         for c in range(CORES)], axis=0)
    new_e = np.concatenate(
        [np.ascontiguousarray(res.results[c]["neweT"].T)
         for c in range(CORES)], axis=0)
    return (new_x, new_e), res


def kernel(**inputs):
    (new_x, new_e), _ = run(inputs, trace=False)
    return new_x, new_e
